# revision 39
# baseline (speedup 1.0000x reference)
"""Fused self-attention (softmax over the QUERY axis) for Trainium2, 8 NeuronCores.

Problem (hardcoded shapes):
    query/key/value: [B=4, S=2048, D=1024] fp32, H=1024
    q = query @ Wq.T + bq ; k = key @ Wk.T + bk ; v = value @ Wv.T + bv
    scores = einsum('bqh,bkh->bqk', q, k) * 0.125
    attn = softmax(scores, axis=1)            # over the QUERY axis
    out  = einsum('bqk,bkh->bqh', attn, v)
    y    = out @ Wo.T + bo

Algebraic restructure (biases bq/bk are zero in this problem's setup_inputs;
a numpy fallback handles the general case):
    scores[q,k] = xq[q,:] @ G @ xk[k,:]^T      with G  = Wq^T @ Wk   [D,D]
    y[q,:]      = sum_k attn[q,k] * vw[k,:]    with vw = (xv @ Gv^T + bvo),
                  Gv = Wo @ Wv [D,D], bvo = Wo @ bv
G / Gv are computed once on the host (fp64), so NO q/k/v/o projections run on
device -- total device work drops to 4 GEMM phases per core:
    P1: M2[d,k]   = sum_e GT[e,d] * xkT[e,k]          (GT = G^T)
    P2: sT[k,q]   = sum_d M2[d,k] * xqT[d,q] ; expT = exp(scale*sT),
                    denom[k] = sum_q expT  (softmax over q needs no max
                    subtraction: |scale*s| <~ 22, well inside fp32 exp range)
    P3: vw[k,d]   = sum_e xvT[e,k] * GvT[e,d] (+bvo) ; vw[k,:] *= 1/denom[k]
    P4: yT[d,q]   = sum_k vw[k,d] * expT[k,q]         (partial over keys)

Sharding: 8 cores = 4 batches x 2 key-halves (T=1024 keys/core). Softmax over
q is per-key, so key-sharding needs no cross-core reduction; the host sums the
two key-half partials of each batch and adds bo.

Performance notes (210us baseline -> ~185us, vs a ~164us pure-matmul floor
of 768 512-wide matmuls at the warm 2.4GHz clock):
  - All operands bf16 (fp8 DoubleRow was measured numerically unusable: the
    attention here is near-argmax, so e4m3's 1.8% rms on dominant weights /
    vw rows gives ~5e-2 scale-relative max error vs the 2e-2 gate; all-bf16
    lands at 9.7e-3 and doubles nothing else). bf16 halves weight-load xbus
    traffic (P2 pace 227 -> 221ns/matmul) and halves input DMA bytes.
  - The NEFF preamble (engine barriers + register TENSOR_LOADs) runs ~7.5us
    before any user instruction; the first few DMAs per queue then pay
    ~1-3us end-to-end latency each, so no real data is consumable before
    ~13.5us. A warmup ladder (12x512-wide + 18x128-wide dummy matmuls)
    spans exactly that window, keeping the PE's HAM clock-gate at the warm
    8/8 setting with zero idle, so P1 flows at full rate from the handoff.
  - P1 runs e-outer (PSUM bank per output chunk) and its gt/xk DMAs are
    issued in consumption order, finest pieces first, alternating queues.
  - expT/vw stored bf16; 1/denom folded into vw inside P3's loop; exp fused
    into P2's PSUM drain via scalar-engine activation with fp32 accum_out.
  - y streamed out per [128,512] tile as each P4 PSUM group closes
    (qb-outer), copies alternating vector/scalar and DMAs alternating both
    queues; the final tile is two 256-wide groups so only ~64KB remains in
    flight after the last matmul.
"""

import numpy as np
import ml_dtypes

import concourse.bacc as bacc
import concourse.bass as bass
import concourse.mybir as mybir
import concourse.tile as tile
from concourse.bass_utils import run_bass_kernel_spmd

P = 128
B = 4
S = 2048          # query sequence length
D = 1024          # embed dim (= hidden dim H)
T = 1024          # keys per core (half of the 2048-key sequence)
DO = D // P       # 8
TO = T // P       # 8
QB = 512          # query block width
NQB = S // QB     # 4
NB = 512
SCALE = 64 ** -0.5
N_WARM = 12

F32 = mybir.dt.float32
F32R = mybir.dt.float32r
BF16 = mybir.dt.bfloat16
AF = mybir.ActivationFunctionType
BF = ml_dtypes.bfloat16


def _build_program():
    nc = bacc.Bacc(None, target_bir_lowering=False)

    xqT = nc.dram_tensor("xqT", [D, S], BF16, kind="ExternalInput")
    xkT = nc.dram_tensor("xkT", [D, T], BF16, kind="ExternalInput")
    xvT = nc.dram_tensor("xvT", [D, T], BF16, kind="ExternalInput")
    gT = nc.dram_tensor("gT", [D, D], BF16, kind="ExternalInput")    # (Wq^T Wk)^T
    gvT = nc.dram_tensor("gvT", [D, D], BF16, kind="ExternalInput")  # (Wo Wv)^T
    bvo = nc.dram_tensor("bvo", [D], F32, kind="ExternalInput")      # Wo @ bv
    y = nc.dram_tensor("y", [D, S], F32, kind="ExternalOutput")      # yT partial

    with tile.TileContext(nc) as tc:
        with (
            tc.tile_pool(name="singles", bufs=1) as singles,
            tc.tile_pool(name="psum", bufs=8, space="PSUM") as psum,
            tc.tile_pool(name="exp_pool", bufs=1) as exp_pool,
            tc.tile_pool(name="work", bufs=1) as work,
            tc.tile_pool(name="xq_pool", bufs=2) as xq_pool,
        ):
            denom = singles.tile([P, TO, NQB], F32, tag="denom")
            dsum = singles.tile([P, TO], F32, tag="dsum")
            recip = singles.tile([P, TO], F32, tag="recip")
            bvo_sb = singles.tile([P, D], F32, tag="bvo")
            bvo_ap = bvo[:]
            # gpsimd (SW DGE) queue: keeps the two fast HW queues free for
            # the gt/xk chunks P1 is waiting on; bvo isn't needed until P3.
            nc.gpsimd.dma_start(
                out=bvo_sb,
                in_=bass.AP(tensor=bvo_ap.tensor, offset=bvo_ap.offset,
                            ap=[[0, P]] + list(bvo_ap.ap)),
            )

            # ---- P1 inputs: gT (sync queue) and the xkT first halves
            # (scalar queue), bf16. The first few DMAs per queue each pay
            # multi-us end-to-end latency regardless of size, so ship FEW
            # LARGE grouped transfers (3D APs over row-blocks), sized so
            # each lands just before P1's e-outer pass consumes it. The xk
            # second halves follow (P1's nb=1 pass needs them much later).
            gt_all = work.tile([P, DO, D], BF16, tag="gta")
            xk_all = work.tile([P, DO, T], BF16, tag="xka")

            def blk_dma(eng, out_ap, dram, row_w, lo, hi, c0, c1):
                base = dram[:]
                eng.dma_start(
                    out=out_ap,
                    in_=bass.AP(tensor=base.tensor,
                                offset=base.offset + lo * P * row_w + c0,
                                ap=[[row_w, P], [P * row_w, hi - lo],
                                    [1, c1 - c0]]),
                )

            # fine first pieces, strictly in consumption order, alternating
            # queues so no needed piece sits deeper than ~1 slot behind
            # another (the early per-DMA latency is ~1-3us regardless of
            # size): sync gets gt0[0:256], gt0[512:], xk1a, gt2, xk3a, ...;
            # scalar gets xk0a, gt0[256:512], gt1, xk2a, gt3, ...
            nc.sync.dma_start(out=gt_all[:, 0, 0:256], in_=gT[0:P, 0:256])
            nc.scalar.dma_start(out=xk_all[:, 0, 0:NB], in_=xkT[0:P, 0:NB])
            nc.sync.dma_start(out=gt_all[:, 0, NB:D], in_=gT[0:P, NB:D])
            nc.scalar.dma_start(out=gt_all[:, 0, 256:512],
                                in_=gT[0:P, 256:512])
            for e in range(1, DO):
                eng_g = nc.scalar if e % 2 == 1 else nc.sync
                eng_x = nc.sync if e % 2 == 1 else nc.scalar
                eng_x.dma_start(out=xk_all[:, e, 0:NB],
                                in_=xkT[e * P:(e + 1) * P, 0:NB])
                eng_g.dma_start(out=gt_all[:, e, :], in_=gT[e * P:(e + 1) * P, :])
            blk_dma(nc.sync, xk_all[:, 0:4, NB:T], xkT, T, 0, 4, NB, T)
            blk_dma(nc.scalar, xk_all[:, 4:8, NB:T], xkT, T, 4, 8, NB, T)

            # first xq block prefetch (bf16): after gt/xk in queue order
            xq_t = [xq_pool.tile([P, DO, QB], BF16, tag="xq", name="xq0")]
            blk_dma(nc.sync, xq_t[0][:, 0:4, :], xqT, S, 0, 4, 0, QB)
            blk_dma(nc.scalar, xq_t[0][:, 4:8, :], xqT, S, 4, 8, 0, QB)

            # HAM warmup: keep the PE busy (~5us at the cold clock) while the
            # first input DMAs land, so real matmuls start at the warm clock.
            # Coarse 512-wide matmuls followed by fine 128-wide ones so the
            # handoff to P1's first real matmul wastes little time.
            wtile = singles.tile([P, NB], BF16, tag="warm")
            nc.vector.memset(wtile.bitcast(F32), 0.0)
            wps = psum.tile([P, NB], F32, tag="ps", name="warm_ps")
            for _ in range(N_WARM):
                nc.tensor.matmul(wps, lhsT=wtile[:, 0:P], rhs=wtile,
                                 start=True, stop=True)
            for _ in range(18):
                nc.tensor.matmul(wps[:, 0:P], lhsT=wtile[:, 0:P],
                                 rhs=wtile[:, 0:P], start=True, stop=True)

            expT = exp_pool.tile([P, TO, S], BF16, tag="expT")  # exp scores [k,q]
            m2 = work.tile([P, DO, T], BF16, tag="m2")          # M2 [d,k]

            # ---- P1: M2[d,k] = sum_e GT[e,d] * xk[e,k], e-outer so the first
            # matmul only needs gt[0]/xk[0]. 8 PSUM banks, one per md chunk.
            for nb in range(T // NB):
                ps1 = [psum.tile([P, NB], F32, tag="ps", name=f"ps_p1_{nb}_{md}")
                       for md in range(DO)]
                for e in range(DO):
                    for md in range(DO):
                        nc.tensor.matmul(
                            ps1[md],
                            lhsT=gt_all[:, e, md * P:(md + 1) * P],
                            rhs=xk_all[:, e, nb * NB:(nb + 1) * NB],
                            start=(e == 0),
                            stop=(e == DO - 1),
                        )
                for md in range(DO):
                    nc.vector.tensor_copy(
                        out=m2[:, md, nb * NB:(nb + 1) * NB], in_=ps1[md]
                    )

            # ---- P2: scores_T -> exp (bf16) + denominators, per query block
            for qb in range(NQB):
                if qb > 0:
                    xq = xq_pool.tile([P, DO, QB], BF16, tag="xq", name=f"xq{qb}")
                    blk_dma(nc.sync, xq[:, 0:4, :], xqT, S, 0, 4,
                            qb * QB, (qb + 1) * QB)
                    blk_dma(nc.scalar, xq[:, 4:8, :], xqT, S, 4, 8,
                            qb * QB, (qb + 1) * QB)
                    xq_t.append(xq)
                xq = xq_t[qb]
                for kt in range(TO):
                    ps = psum.tile([P, QB], F32, tag="ps")
                    for d in range(DO):
                        nc.tensor.matmul(
                            ps,
                            lhsT=m2[:, d, kt * P:(kt + 1) * P],
                            rhs=xq[:, d, :],
                            start=(d == 0),
                            stop=(d == DO - 1),
                        )
                    nc.scalar.activation(
                        out=expT[:, kt, qb * QB:(qb + 1) * QB],
                        in_=ps,
                        func=AF.Exp,
                        scale=float(SCALE),
                        accum_out=denom[:, kt, qb:qb + 1],
                    )
                if qb == 0:
                    # P3 inputs: xvT reuses the gT slot (sync), GvT reuses
                    # the xkT slot (scalar); queued here to stream during P2.
                    xv_all = work.tile([P, DO, T], BF16, tag="gta", name="xv")
                    gv_all = work.tile([P, DO, D], BF16, tag="xka", name="gv")
                    for lo, hi in ((0, 4), (4, 8)):
                        blk_dma(nc.sync, xv_all[:, lo:hi, :], xvT, T,
                                lo, hi, 0, T)
                        blk_dma(nc.scalar, gv_all[:, lo:hi, :], gvT, D,
                                lo, hi, 0, D)

            # ---- softmax denominators -> 1/denom per key
            nc.vector.reduce_sum(out=dsum, in_=denom, axis=mybir.AxisListType.X)
            nc.vector.reciprocal(out=recip, in_=dsum)

            # ---- P3: vw[k,d] = (sum_e xv[e,k] * GvT[e,d] + bvo) / denom[k]
            # bias-add and 1/denom fold interleaved per mk chunk (bf16 out)
            vw = work.tile([P, TO, D], BF16, tag="m2")  # reuses M2's slot
            for mk in range(TO):
                ps3 = [psum.tile([P, NB], F32, tag="ps", name=f"ps_p3_{mk}_{i}")
                       for i in range(D // NB)]
                for e in range(DO):
                    for nb in range(D // NB):
                        nc.tensor.matmul(
                            ps3[nb],
                            lhsT=xv_all[:, e, mk * P:(mk + 1) * P],
                            rhs=gv_all[:, e, nb * NB:(nb + 1) * NB],
                            start=(e == 0),
                            stop=(e == DO - 1),
                        )
                for nb in range(D // NB):
                    nc.vector.tensor_add(
                        out=vw[:, mk, nb * NB:(nb + 1) * NB],
                        in0=ps3[nb],
                        in1=bvo_sb[:, nb * NB:(nb + 1) * NB],
                    )
                nc.vector.tensor_scalar_mul(
                    out=vw[:, mk, :], in0=vw[:, mk, :], scalar1=recip[:, mk:mk + 1]
                )

            # ---- P4: yT[d,q] = sum_k vw[k,d] * expT[k,q] (partial over keys)
            # qb-outer: each [128,512] output tile completes as its 8-matmul
            # group ends, so copies + y DMAs drain while later tiles compute;
            # after the very last matmul only one 0.25MB transfer remains.
            for md in range(DO):
                yt = xq_pool.tile([P, S], F32, tag="xq")  # reuses xq slots
                for qb in range(NQB):
                    if md == DO - 1 and qb == NQB - 1:
                        # very last output tile in 256/128/128-wide PSUM
                        # groups: the earlier pieces copy + stream while the
                        # later groups still accumulate, so only ~32KB is in
                        # flight after the very last matmul
                        pieces = [(0, 256, "v", nc.sync),
                                  (256, 384, "s", nc.scalar),
                                  (384, 512, "v", nc.sync)]
                        for h, (a, b, cp, eng) in enumerate(pieces):
                            c0, c1 = qb * QB + a, qb * QB + b
                            psh = psum.tile([P, b - a], F32, tag="ps",
                                            name=f"ps_p4_last_{h}")
                            for kt in range(TO):
                                nc.tensor.matmul(
                                    psh,
                                    lhsT=vw[:, kt, md * P:(md + 1) * P],
                                    rhs=expT[:, kt, c0:c1],
                                    start=(kt == 0),
                                    stop=(kt == TO - 1),
                                )
                            if cp == "v":
                                nc.vector.tensor_copy(
                                    out=yt[:, c0:c1], in_=psh
                                )
                            else:
                                nc.scalar.copy(out=yt[:, c0:c1], in_=psh)
                            eng.dma_start(
                                out=y[md * P:(md + 1) * P, c0:c1],
                                in_=yt[:, c0:c1],
                            )
                        continue
                    ps4 = psum.tile([P, QB], F32, tag="ps",
                                    name=f"ps_p4_{md}_{qb}")
                    for kt in range(TO):
                        nc.tensor.matmul(
                            ps4,
                            lhsT=vw[:, kt, md * P:(md + 1) * P],
                            rhs=expT[:, kt, qb * QB:(qb + 1) * QB],
                            start=(kt == 0),
                            stop=(kt == TO - 1),
                        )
                    if qb % 2 == 0:
                        nc.vector.tensor_copy(
                            out=yt[:, qb * QB:(qb + 1) * QB], in_=ps4
                        )
                    else:
                        nc.scalar.copy(
                            out=yt[:, qb * QB:(qb + 1) * QB], in_=ps4
                        )
                    eng = nc.sync if (md + qb) % 2 == 0 else nc.scalar
                    eng.dma_start(
                        out=y[md * P:(md + 1) * P, qb * QB:(qb + 1) * QB],
                        in_=yt[:, qb * QB:(qb + 1) * QB],
                    )

    nc.finalize()
    return nc


_NC_CACHE = []


def _get_nc():
    if not _NC_CACHE:
        _NC_CACHE.append(_build_program())
    return _NC_CACHE[0]


def _numpy_fallback(query, key, value, Wq, bq, Wk, bk, Wv, bv, Wo, bo):
    f = np.float32
    q = np.einsum("bsd,hd->bsh", query, Wq).astype(f) + bq
    k = np.einsum("bsd,hd->bsh", key, Wk).astype(f) + bk
    v = np.einsum("bsd,hd->bsh", value, Wv).astype(f) + bv
    s = np.einsum("bqh,bkh->bqk", q, k) * np.float32(SCALE)
    s = s - s.max(axis=1, keepdims=True)
    e = np.exp(s)
    attn = e / e.sum(axis=1, keepdims=True)
    out = np.einsum("bqk,bkh->bqh", attn, v)
    return (np.einsum("bqh,dh->bqd", out, Wo) + bo).astype(f)


def run(query, key, value, Wq, bq, Wk, bk, Wv, bv, Wo, bo, **spmd_kwargs):
    """Run on 8 cores; returns (output [B,S,D] fp32, BassKernelResults|None)."""
    f = np.float32
    query = np.asarray(query, f)
    key = np.asarray(key, f)
    value = np.asarray(value, f)
    Wq, Wk, Wv, Wo = (np.asarray(w, f) for w in (Wq, Wk, Wv, Wo))
    bq, bk, bv, bo = (np.asarray(b_, f) for b_ in (bq, bk, bv, bo))

    if np.any(bq) or np.any(bk):
        # The G-composition absorbs the q/k projections and cannot represent
        # nonzero q/k biases; this problem's setup_inputs always has zeros.
        return _numpy_fallback(query, key, value, Wq, bq, Wk, bk, Wv, bv, Wo, bo), None

    w64 = np.float64
    gT = (Wk.astype(w64).T @ Wq.astype(w64)).astype(f)   # G^T
    gvT = (Wv.astype(w64).T @ Wo.astype(w64).T).astype(f)
    bvo = (Wo.astype(w64) @ bv.astype(w64)).astype(f)
    gT_bf = np.ascontiguousarray(gT.astype(BF))
    gvT_bf = np.ascontiguousarray(gvT.astype(BF))

    in_maps = []
    for core in range(8):
        b, half = divmod(core, 2)
        sl = slice(half * T, (half + 1) * T)
        in_maps.append({
            "xqT": np.ascontiguousarray(query[b].T.astype(BF)),      # [D, S]
            "xkT": np.ascontiguousarray(key[b, sl].T.astype(BF)),    # [D, T]
            "xvT": np.ascontiguousarray(value[b, sl].T.astype(BF)),  # [D, T]
            "gT": gT_bf, "gvT": gvT_bf, "bvo": bvo,
        })

    nc = _get_nc()
    res = run_bass_kernel_spmd(nc, in_maps, core_ids=list(range(8)), **spmd_kwargs)
    out = np.stack(
        [(res.results[2 * b]["y"] + res.results[2 * b + 1]["y"]).T + bo
         for b in range(B)]
    ).astype(f)
    return out, res


def kernel(query, key, value, Wq, bq, Wk, bk, Wv, bv, Wo, bo):
    out, _ = run(query, key, value, Wq, bq, Wk, bk, Wv, bv, Wo, bo)
    return out


# revision 40
# speedup vs baseline: 1.0029x; 1.0029x over previous
"""Fused self-attention (softmax over the QUERY axis) for Trainium2, 8 NeuronCores.

Problem (hardcoded shapes):
    query/key/value: [B=4, S=2048, D=1024] fp32, H=1024
    q = query @ Wq.T + bq ; k = key @ Wk.T + bk ; v = value @ Wv.T + bv
    scores = einsum('bqh,bkh->bqk', q, k) * 0.125
    attn = softmax(scores, axis=1)            # over the QUERY axis
    out  = einsum('bqk,bkh->bqh', attn, v)
    y    = out @ Wo.T + bo

Algebraic restructure (biases bq/bk are zero in this problem's setup_inputs;
a numpy fallback handles the general case):
    scores[q,k] = xq[q,:] @ G @ xk[k,:]^T      with G  = Wq^T @ Wk   [D,D]
    y[q,:]      = sum_k attn[q,k] * vw[k,:]    with vw = (xv @ Gv^T + bvo),
                  Gv = Wo @ Wv [D,D], bvo = Wo @ bv
G / Gv are computed once on the host (fp64), so NO q/k/v/o projections run on
device -- total device work drops to 4 GEMM phases per core:
    P1: M2[d,k]   = sum_e GT[e,d] * xkT[e,k]          (GT = G^T)
    P2: sT[k,q]   = sum_d M2[d,k] * xqT[d,q] ; expT = exp(scale*sT),
                    denom[k] = sum_q expT  (softmax over q needs no max
                    subtraction: |scale*s| <~ 22, well inside fp32 exp range)
    P3: vw[k,d]   = sum_e xvT[e,k] * GvT[e,d] (+bvo) ; vw[k,:] *= 1/denom[k]
    P4: yT[d,q]   = sum_k vw[k,d] * expT[k,q]         (partial over keys)

Sharding: 8 cores = 4 batches x 2 key-halves (T=1024 keys/core). Softmax over
q is per-key, so key-sharding needs no cross-core reduction; the host sums the
two key-half partials of each batch and adds bo.

Performance notes (210us baseline -> ~185us, vs a ~164us pure-matmul floor
of 768 512-wide matmuls at the warm 2.4GHz clock):
  - All operands bf16 (fp8 DoubleRow was measured numerically unusable: the
    attention here is near-argmax, so e4m3's 1.8% rms on dominant weights /
    vw rows gives ~5e-2 scale-relative max error vs the 2e-2 gate; all-bf16
    lands at 9.7e-3 and doubles nothing else). bf16 halves weight-load xbus
    traffic (P2 pace 227 -> 221ns/matmul) and halves input DMA bytes.
  - The NEFF preamble (engine barriers + register TENSOR_LOADs) runs ~7.5us
    before any user instruction; the first few DMAs per queue then pay
    ~1-3us end-to-end latency each, so no real data is consumable before
    ~13.5us. A warmup ladder (12x512-wide + 18x128-wide dummy matmuls)
    spans exactly that window, keeping the PE's HAM clock-gate at the warm
    8/8 setting with zero idle, so P1 flows at full rate from the handoff.
  - P1 runs e-outer (PSUM bank per output chunk) and its gt/xk DMAs are
    issued in consumption order, finest pieces first, alternating queues.
  - expT/vw stored bf16; 1/denom folded into vw inside P3's loop; exp fused
    into P2's PSUM drain via scalar-engine activation with fp32 accum_out.
  - y streamed out per [128,512] tile as each P4 PSUM group closes
    (qb-outer), copies alternating vector/scalar and DMAs alternating both
    queues; the final tile is two 256-wide groups so only ~64KB remains in
    flight after the last matmul.
"""

import numpy as np
import ml_dtypes

import concourse.bacc as bacc
import concourse.bass as bass
import concourse.mybir as mybir
import concourse.tile as tile
from concourse.bass_utils import run_bass_kernel_spmd

P = 128
B = 4
S = 2048          # query sequence length
D = 1024          # embed dim (= hidden dim H)
T = 1024          # keys per core (half of the 2048-key sequence)
DO = D // P       # 8
TO = T // P       # 8
QB = 512          # query block width
NQB = S // QB     # 4
NB = 512
SCALE = 64 ** -0.5
N_WARM = 12

F32 = mybir.dt.float32
F32R = mybir.dt.float32r
BF16 = mybir.dt.bfloat16
AF = mybir.ActivationFunctionType
BF = ml_dtypes.bfloat16


def _build_program():
    nc = bacc.Bacc(None, target_bir_lowering=False)

    xqT = nc.dram_tensor("xqT", [D, S], BF16, kind="ExternalInput")
    xkT = nc.dram_tensor("xkT", [D, T], BF16, kind="ExternalInput")
    xvT = nc.dram_tensor("xvT", [D, T], BF16, kind="ExternalInput")
    gT = nc.dram_tensor("gT", [D, D], BF16, kind="ExternalInput")    # (Wq^T Wk)^T
    gvT = nc.dram_tensor("gvT", [D, D], BF16, kind="ExternalInput")  # (Wo Wv)^T
    bvo = nc.dram_tensor("bvo", [D], F32, kind="ExternalInput")      # Wo @ bv
    y = nc.dram_tensor("y", [D, S], F32, kind="ExternalOutput")      # yT partial

    with tile.TileContext(nc) as tc:
        with (
            tc.tile_pool(name="singles", bufs=1) as singles,
            tc.tile_pool(name="psum", bufs=8, space="PSUM") as psum,
            tc.tile_pool(name="exp_pool", bufs=1) as exp_pool,
            tc.tile_pool(name="work", bufs=1) as work,
            tc.tile_pool(name="xq_pool", bufs=2) as xq_pool,
        ):
            denom = singles.tile([P, TO, NQB], F32, tag="denom")
            dsum = singles.tile([P, TO], F32, tag="dsum")
            recip = singles.tile([P, TO], F32, tag="recip")
            bvo_sb = singles.tile([P, D], F32, tag="bvo")
            bvo_ap = bvo[:]
            # gpsimd (SW DGE) queue: keeps the two fast HW queues free for
            # the gt/xk chunks P1 is waiting on; bvo isn't needed until P3.
            nc.gpsimd.dma_start(
                out=bvo_sb,
                in_=bass.AP(tensor=bvo_ap.tensor, offset=bvo_ap.offset,
                            ap=[[0, P]] + list(bvo_ap.ap)),
            )

            # ---- P1 inputs: gT (sync queue) and the xkT first halves
            # (scalar queue), bf16. The first few DMAs per queue each pay
            # multi-us end-to-end latency regardless of size, so ship FEW
            # LARGE grouped transfers (3D APs over row-blocks), sized so
            # each lands just before P1's e-outer pass consumes it. The xk
            # second halves follow (P1's nb=1 pass needs them much later).
            gt_all = work.tile([P, DO, D], BF16, tag="gta")
            xk_all = work.tile([P, DO, T], BF16, tag="xka")

            def blk_dma(eng, out_ap, dram, row_w, lo, hi, c0, c1):
                base = dram[:]
                eng.dma_start(
                    out=out_ap,
                    in_=bass.AP(tensor=base.tensor,
                                offset=base.offset + lo * P * row_w + c0,
                                ap=[[row_w, P], [P * row_w, hi - lo],
                                    [1, c1 - c0]]),
                )

            # fine first pieces, strictly in consumption order, alternating
            # queues so no needed piece sits deeper than ~1 slot behind
            # another (the early per-DMA latency is ~1-3us regardless of
            # size): sync gets gt0[0:256], gt0[512:], xk1a, gt2, xk3a, ...;
            # scalar gets xk0a, gt0[256:512], gt1, xk2a, gt3, ...
            nc.sync.dma_start(out=gt_all[:, 0, 0:256], in_=gT[0:P, 0:256])
            nc.scalar.dma_start(out=xk_all[:, 0, 0:NB], in_=xkT[0:P, 0:NB])
            nc.sync.dma_start(out=gt_all[:, 0, NB:D], in_=gT[0:P, NB:D])
            nc.scalar.dma_start(out=gt_all[:, 0, 256:512],
                                in_=gT[0:P, 256:512])
            for e in range(1, DO):
                eng_g = nc.scalar if e % 2 == 1 else nc.sync
                eng_x = nc.sync if e % 2 == 1 else nc.scalar
                eng_x.dma_start(out=xk_all[:, e, 0:NB],
                                in_=xkT[e * P:(e + 1) * P, 0:NB])
                eng_g.dma_start(out=gt_all[:, e, :], in_=gT[e * P:(e + 1) * P, :])
            blk_dma(nc.sync, xk_all[:, 0:4, NB:T], xkT, T, 0, 4, NB, T)
            blk_dma(nc.scalar, xk_all[:, 4:8, NB:T], xkT, T, 4, 8, NB, T)

            # first xq block prefetch (bf16): after gt/xk in queue order
            xq_t = [xq_pool.tile([P, DO, QB], BF16, tag="xq", name="xq0")]
            blk_dma(nc.sync, xq_t[0][:, 0:4, :], xqT, S, 0, 4, 0, QB)
            blk_dma(nc.scalar, xq_t[0][:, 4:8, :], xqT, S, 4, 8, 0, QB)

            # HAM warmup: keep the PE busy (~5us at the cold clock) while the
            # first input DMAs land, so real matmuls start at the warm clock.
            # Coarse 512-wide matmuls followed by fine 128-wide ones so the
            # handoff to P1's first real matmul wastes little time.
            wtile = singles.tile([P, NB], BF16, tag="warm")
            nc.vector.memset(wtile.bitcast(F32), 0.0)
            wps = psum.tile([P, NB], F32, tag="ps", name="warm_ps")
            for _ in range(N_WARM):
                nc.tensor.matmul(wps, lhsT=wtile[:, 0:P], rhs=wtile,
                                 start=True, stop=True)
            for _ in range(18):
                nc.tensor.matmul(wps[:, 0:P], lhsT=wtile[:, 0:P],
                                 rhs=wtile[:, 0:P], start=True, stop=True)

            expT = exp_pool.tile([P, TO, S], BF16, tag="expT")  # exp scores [k,q]
            m2 = work.tile([P, DO, T], BF16, tag="m2")          # M2 [d,k]

            # ---- P1: M2[d,k] = sum_e GT[e,d] * xk[e,k], e-outer so the first
            # matmul only needs gt[0]/xk[0]. 8 PSUM banks, one per md chunk.
            for nb in range(T // NB):
                ps1 = [psum.tile([P, NB], F32, tag="ps", name=f"ps_p1_{nb}_{md}")
                       for md in range(DO)]
                for e in range(DO):
                    for md in range(DO):
                        nc.tensor.matmul(
                            ps1[md],
                            lhsT=gt_all[:, e, md * P:(md + 1) * P],
                            rhs=xk_all[:, e, nb * NB:(nb + 1) * NB],
                            start=(e == 0),
                            stop=(e == DO - 1),
                        )
                for md in range(DO):
                    nc.vector.tensor_copy(
                        out=m2[:, md, nb * NB:(nb + 1) * NB], in_=ps1[md]
                    )

            # ---- P2: scores_T -> exp (bf16) + denominators, per query block
            for qb in range(NQB):
                if qb > 0:
                    xq = xq_pool.tile([P, DO, QB], BF16, tag="xq", name=f"xq{qb}")
                    blk_dma(nc.sync, xq[:, 0:4, :], xqT, S, 0, 4,
                            qb * QB, (qb + 1) * QB)
                    blk_dma(nc.scalar, xq[:, 4:8, :], xqT, S, 4, 8,
                            qb * QB, (qb + 1) * QB)
                    xq_t.append(xq)
                xq = xq_t[qb]
                for kt in range(TO):
                    ps = psum.tile([P, QB], F32, tag="ps")
                    for d in range(DO):
                        nc.tensor.matmul(
                            ps,
                            lhsT=m2[:, d, kt * P:(kt + 1) * P],
                            rhs=xq[:, d, :],
                            start=(d == 0),
                            stop=(d == DO - 1),
                        )
                    nc.scalar.activation(
                        out=expT[:, kt, qb * QB:(qb + 1) * QB],
                        in_=ps,
                        func=AF.Exp,
                        scale=float(SCALE),
                        accum_out=denom[:, kt, qb:qb + 1],
                    )
                if qb == 0:
                    # P3 inputs: xvT reuses the gT slot (sync), GvT reuses
                    # the xkT slot (scalar); queued here to stream during P2.
                    xv_all = work.tile([P, DO, T], BF16, tag="gta", name="xv")
                    gv_all = work.tile([P, DO, D], BF16, tag="xka", name="gv")
                    for lo, hi in ((0, 4), (4, 8)):
                        blk_dma(nc.sync, xv_all[:, lo:hi, :], xvT, T,
                                lo, hi, 0, T)
                        blk_dma(nc.scalar, gv_all[:, lo:hi, :], gvT, D,
                                lo, hi, 0, D)

            # ---- softmax denominators -> 1/denom per key
            nc.vector.reduce_sum(out=dsum, in_=denom, axis=mybir.AxisListType.X)
            nc.vector.reciprocal(out=recip, in_=dsum)

            # ---- P3: vw[k,d] = (sum_e xv[e,k] * GvT[e,d] + bvo) / denom[k]
            # bias-add and 1/denom fold interleaved per mk chunk (bf16 out)
            vw = work.tile([P, TO, D], BF16, tag="m2")  # reuses M2's slot
            for mk in range(TO):
                ps3 = [psum.tile([P, NB], F32, tag="ps", name=f"ps_p3_{mk}_{i}")
                       for i in range(D // NB)]
                for e in range(DO):
                    for nb in range(D // NB):
                        nc.tensor.matmul(
                            ps3[nb],
                            lhsT=xv_all[:, e, mk * P:(mk + 1) * P],
                            rhs=gv_all[:, e, nb * NB:(nb + 1) * NB],
                            start=(e == 0),
                            stop=(e == DO - 1),
                        )
                for nb in range(D // NB):
                    nc.vector.tensor_add(
                        out=vw[:, mk, nb * NB:(nb + 1) * NB],
                        in0=ps3[nb],
                        in1=bvo_sb[:, nb * NB:(nb + 1) * NB],
                    )
                nc.vector.tensor_scalar_mul(
                    out=vw[:, mk, :], in0=vw[:, mk, :], scalar1=recip[:, mk:mk + 1]
                )

            # ---- P4: yT[d,q] = sum_k vw[k,d] * expT[k,q] (partial over keys)
            # qb-outer: each [128,512] output tile completes as its 8-matmul
            # group ends, so copies + y DMAs drain while later tiles compute;
            # after the very last matmul only one 0.25MB transfer remains.
            for md in range(DO):
                yt = xq_pool.tile([P, S], F32, tag="xq")  # reuses xq slots
                for qb in range(NQB):
                    if md == DO - 1 and qb == NQB - 1:
                        # very last output tile: two 256-wide PSUM groups,
                        # each copied + DMAed on its own engine/queue, so
                        # only ~64KB remains in flight after the final matmul
                        for h in range(2):
                            c0 = qb * QB + h * (QB // 2)
                            psh = psum.tile([P, QB // 2], F32, tag="ps",
                                            name=f"ps_p4_last_{h}")
                            for kt in range(TO):
                                nc.tensor.matmul(
                                    psh,
                                    lhsT=vw[:, kt, md * P:(md + 1) * P],
                                    rhs=expT[:, kt, c0:c0 + QB // 2],
                                    start=(kt == 0),
                                    stop=(kt == TO - 1),
                                )
                            if h == 0:
                                nc.vector.tensor_copy(
                                    out=yt[:, c0:c0 + QB // 2], in_=psh
                                )
                            else:
                                nc.scalar.copy(
                                    out=yt[:, c0:c0 + QB // 2], in_=psh
                                )
                            eng = nc.sync if h == 0 else nc.scalar
                            eng.dma_start(
                                out=y[md * P:(md + 1) * P, c0:c0 + QB // 2],
                                in_=yt[:, c0:c0 + QB // 2],
                            )
                        continue
                    ps4 = psum.tile([P, QB], F32, tag="ps",
                                    name=f"ps_p4_{md}_{qb}")
                    for kt in range(TO):
                        nc.tensor.matmul(
                            ps4,
                            lhsT=vw[:, kt, md * P:(md + 1) * P],
                            rhs=expT[:, kt, qb * QB:(qb + 1) * QB],
                            start=(kt == 0),
                            stop=(kt == TO - 1),
                        )
                    if qb % 2 == 0:
                        nc.vector.tensor_copy(
                            out=yt[:, qb * QB:(qb + 1) * QB], in_=ps4
                        )
                    else:
                        nc.scalar.copy(
                            out=yt[:, qb * QB:(qb + 1) * QB], in_=ps4
                        )
                    eng = nc.sync if (md + qb) % 2 == 0 else nc.scalar
                    eng.dma_start(
                        out=y[md * P:(md + 1) * P, qb * QB:(qb + 1) * QB],
                        in_=yt[:, qb * QB:(qb + 1) * QB],
                    )

    nc.finalize()
    return nc


_NC_CACHE = []


def _get_nc():
    if not _NC_CACHE:
        _NC_CACHE.append(_build_program())
    return _NC_CACHE[0]


def _numpy_fallback(query, key, value, Wq, bq, Wk, bk, Wv, bv, Wo, bo):
    f = np.float32
    q = np.einsum("bsd,hd->bsh", query, Wq).astype(f) + bq
    k = np.einsum("bsd,hd->bsh", key, Wk).astype(f) + bk
    v = np.einsum("bsd,hd->bsh", value, Wv).astype(f) + bv
    s = np.einsum("bqh,bkh->bqk", q, k) * np.float32(SCALE)
    s = s - s.max(axis=1, keepdims=True)
    e = np.exp(s)
    attn = e / e.sum(axis=1, keepdims=True)
    out = np.einsum("bqk,bkh->bqh", attn, v)
    return (np.einsum("bqh,dh->bqd", out, Wo) + bo).astype(f)


def run(query, key, value, Wq, bq, Wk, bk, Wv, bv, Wo, bo, **spmd_kwargs):
    """Run on 8 cores; returns (output [B,S,D] fp32, BassKernelResults|None)."""
    f = np.float32
    query = np.asarray(query, f)
    key = np.asarray(key, f)
    value = np.asarray(value, f)
    Wq, Wk, Wv, Wo = (np.asarray(w, f) for w in (Wq, Wk, Wv, Wo))
    bq, bk, bv, bo = (np.asarray(b_, f) for b_ in (bq, bk, bv, bo))

    if np.any(bq) or np.any(bk):
        # The G-composition absorbs the q/k projections and cannot represent
        # nonzero q/k biases; this problem's setup_inputs always has zeros.
        return _numpy_fallback(query, key, value, Wq, bq, Wk, bk, Wv, bv, Wo, bo), None

    w64 = np.float64
    gT = (Wk.astype(w64).T @ Wq.astype(w64)).astype(f)   # G^T
    gvT = (Wv.astype(w64).T @ Wo.astype(w64).T).astype(f)
    bvo = (Wo.astype(w64) @ bv.astype(w64)).astype(f)
    gT_bf = np.ascontiguousarray(gT.astype(BF))
    gvT_bf = np.ascontiguousarray(gvT.astype(BF))

    in_maps = []
    for core in range(8):
        b, half = divmod(core, 2)
        sl = slice(half * T, (half + 1) * T)
        in_maps.append({
            "xqT": np.ascontiguousarray(query[b].T.astype(BF)),      # [D, S]
            "xkT": np.ascontiguousarray(key[b, sl].T.astype(BF)),    # [D, T]
            "xvT": np.ascontiguousarray(value[b, sl].T.astype(BF)),  # [D, T]
            "gT": gT_bf, "gvT": gvT_bf, "bvo": bvo,
        })

    nc = _get_nc()
    res = run_bass_kernel_spmd(nc, in_maps, core_ids=list(range(8)), **spmd_kwargs)
    out = np.stack(
        [(res.results[2 * b]["y"] + res.results[2 * b + 1]["y"]).T + bo
         for b in range(B)]
    ).astype(f)
    return out, res


def kernel(query, key, value, Wq, bq, Wk, bk, Wv, bv, Wo, bo):
    out, _ = run(query, key, value, Wq, bq, Wk, bk, Wv, bv, Wo, bo)
    return out
